# revision 3
# baseline (speedup 1.0000x reference)
"""SMAQBlockVQ kernel for trn2 (8 NeuronCores, SPMD over tokens).

Self-contained: argmax custom-DVE op + host packing + bass kernel + unscramble.

Pipeline per 128-token tile (tokens on PSUM partitions):
  PE  : block-diagonal scores = 2*x.M - |c|^2 via W [65,2048]x2 (bias row folded)
  ACT : evacuate scores PSUM->SBUF fp32
  DVE : ARGMAX2_ANT fused custom op - single pass, two interleaved streams,
        exact fp32 argmax with index packed via running-max hits
  GPSIMD ap_gather: decode table lookup, idx tensor = flat indices directly
  host: final layout permutation (unscramble)
"""

import numpy as np

import concourse.bacc as bacc
import concourse.mybir as mybir
import concourse.tile as tile
import concourse.bass as bass
from concourse import bass_utils

N_CORES = 8
B, H, S, DDIM = 4, 16, 4096, 128
NB, BD, NCENT = 16, 8, 256
N_TOK = B * H * S
T_CORE = N_TOK // N_CORES
TILE = 128
N_TILES = T_CORE // TILE
F32 = mybir.dt.float32
I32 = mybir.dt.int32
I16 = mybir.dt.int16

import numpy as np

from concourse.dve_spec import (
    Spec, Src0, Src1, C1, C2, Zero, AluOp, maxx, eq, lower, Scan,
)
from concourse.dve_uop import DveOpSpec, DveVer
from concourse import dve_ops
from concourse.dve_ops import DveOp, OPS, CUSTOM_DVE_SPECS, _SUB_OPCODE_FOR_NAME, _CUSTOM_DVE_ROW_BASE


def _ref(in0, in1, s0, s1, imm2):
    mk = np.maximum(in0, in1)
    r = np.maximum.accumulate(mk, axis=-1)
    hit = (mk == r).astype(np.float32)
    side = (mk == in1).astype(np.float32)
    n = in0.shape[-1]
    i2 = (s1 + imm2 * np.arange(n, dtype=np.float32)).astype(np.float32)
    val = hit * (i2 + side)
    return val, np.max(val, axis=-1, keepdims=True)


def _make_spec():
    mk = maxx(Src0, Src1)
    r = Scan(AluOp.MAX, mk)
    hit = eq(mk, r)
    side = eq(mk, Src1)
    i2 = Scan(AluOp.ADD, C2, init=C1 - C2)
    val = hit * (i2 + side)
    return Spec(body=val, accum=AluOp.MAX, accum_init=Zero, reference=_ref)


ARGMAX2 = None


def register():
    global ARGMAX2
    if ARGMAX2 is not None:
        return ARGMAX2
    spec = _make_spec()
    # compute the uops sha for both hwdecode generations so the pin check passes
    shas = {}
    for ver in ("v3", "v4"):
        uops = lower(spec, ver=ver)
        shas[ver] = DveOpSpec(name="ARGMAX2_ANT", opcode=0, uops=uops,
                              rd1_en=True).sha(ver)
    op = DveOp("ARGMAX2_ANT", spec, subdim=False, uops_sha=shas)
    OPS.append(op)
    CUSTOM_DVE_SPECS[op.name] = op.spec
    _SUB_OPCODE_FOR_NAME[op.name] = _CUSTOM_DVE_ROW_BASE + len(OPS) - 1
    assert _SUB_OPCODE_FOR_NAME[op.name] < 0x20
    ARGMAX2 = op
    return op




def _base_prep(k, E_blocks, centroids, decoded_centroids, n_tiles=N_TILES):
    """Host-side packing. Returns per-core in_maps."""
    k = np.asarray(k, dtype=np.float32)
    E = np.asarray(E_blocks, dtype=np.float64)
    C = np.asarray(centroids, dtype=np.float64)
    D = np.asarray(decoded_centroids, dtype=np.float32)

    M = np.einsum('bce,bed->bcd', C, E)          # (16, 256, 8)
    c2 = np.sum(C * C, axis=-1)                  # (16, 256)
    W = np.zeros((2, 65, 2048), np.float64)
    for h in range(2):
        for bl in range(8):
            b = 8 * h + bl
            W[h, 8 * bl:8 * bl + 8, 256 * bl:256 * (bl + 1)] = 2.0 * M[b].T
            W[h, 64, 256 * bl:256 * (bl + 1)] = -c2[b]
    W = W.astype(np.float32)

    t_core = n_tiles * TILE
    x = k.reshape(-1, DDIM)
    xr = x[:N_CORES * t_core].reshape(N_CORES, n_tiles, TILE, DDIM)
    kaug = np.ones((N_CORES, 2, n_tiles, 65, TILE), np.float32)
    kaug[:, 0, :, 0:64, :] = xr[..., 0:64].transpose(0, 1, 3, 2)
    kaug[:, 1, :, 0:64, :] = xr[..., 64:128].transpose(0, 1, 3, 2)

    iota = np.broadcast_to(np.arange(1, 257, dtype=np.float32), (128, 256)).copy()
    binc = np.broadcast_to((256.0 * np.arange(16) - 1.0).astype(np.float32), (128, 16)).copy()
    dec = np.ascontiguousarray(D.reshape(NB * NCENT, BD))

    in_maps = []
    for c in range(N_CORES):
        in_maps.append({
            "kaug": np.ascontiguousarray(kaug[c]),
            "w": W, "iota": iota, "binc": binc, "dec": dec,
        })
    return in_maps




_NC_CACHE = {}
C1BIG = 4096.0


def build_nc(n_tiles=N_TILES, repeat=1):
    key = (n_tiles, repeat)
    if key in _NC_CACHE:
        return _NC_CACHE[key]
    op = register()
    nc = bacc.Bacc("TRN2", target_bir_lowering=False, debug=False,
                   enable_asserts=False, num_devices=N_CORES)

    kaug_d = nc.dram_tensor("kaug", [2, n_tiles, 65, TILE], F32, kind="ExternalInput").ap()
    w_d = nc.dram_tensor("w", [2, 65, 2048], F32, kind="ExternalInput").ap()
    binc_d = nc.dram_tensor("binc", [128, 16], F32, kind="ExternalInput").ap()
    dect_d = nc.dram_tensor("dect", [128, NB * NCENT], F32, kind="ExternalInput").ap()
    out_d = nc.dram_tensor("khat", [n_tiles, 128, 256], F32, kind="ExternalOutput").ap()

    with tile.TileContext(nc) as tc:
        with (
            tc.tile_pool(name="const", bufs=1) as cpool,
            tc.tile_pool(name="xin", bufs=4) as xpool,
            tc.tile_pool(name="ps", bufs=4, space="PSUM") as pspool,
            tc.tile_pool(name="sc", bufs=4) as scpool,
            tc.tile_pool(name="scr", bufs=2) as scrpool,
            tc.tile_pool(name="small", bufs=3) as spool,
            tc.tile_pool(name="gd", bufs=3) as gdpool,
        ):
            w_sb = cpool.tile([65, 2 * 2048], F32, tag="w")
            nc.sync.dma_start(w_sb[:, 0:2048], w_d[0])
            nc.sync.dma_start(w_sb[:, 2048:4096], w_d[1])
            binc_sb = cpool.tile([128, 16], F32, tag="binc")
            nc.sync.dma_start(binc_sb[:], binc_d[:])
            dect_sb = cpool.tile([128, NB * NCENT], F32, tag="dect")
            nc.sync.dma_start(dect_sb[:], dect_d[:])

            def tile_body(t):
                am = spool.tile([128, 16], F32, tag="am")
                xh = [None, None]
                for h in range(2):
                    xh[h] = xpool.tile([65, TILE], F32, tag=f"x{h}", name=f"x{h}")
                    nc.sync.dma_start(xh[h][:], kaug_d[h, t])
                for q in range(4):
                    h = q // 2
                    p = pspool.tile([128, 1024], F32, tag="ps", name="ps")
                    for j in range(2):
                        base = h * 2048 + (q % 2) * 1024 + j * 512
                        nc.tensor.matmul(
                            out=p[:, j * 512:(j + 1) * 512],
                            lhsT=xh[h][:],
                            rhs=w_sb[:, base:base + 512],
                            start=True, stop=True)
                    sc = scpool.tile([128, 1024], F32, tag="sc", name="sc")
                    nc.scalar.copy(out=sc[:], in_=p[:])
                    v = sc[:].rearrange("p (b k two) -> p b k two", b=4, two=2)
                    for bl in range(4):
                        b = q * 4 + bl
                        scr = scrpool.tile([128, 128], F32, tag="scr", name="scr")
                        nc.vector._custom_dve(
                            op, out=scr[:],
                            in0=v[:, bl, :, 0], in1=v[:, bl, :, 1],
                            s1=C1BIG, imm2=2.0,
                            accum_out=am[:, b:b + 1])

                flat_f = spool.tile([128, 16], F32, tag="flatf")
                nc.vector.tensor_tensor(out=flat_f[:], in0=am[:], in1=binc_sb[:],
                                        op=mybir.AluOpType.add)
                flat_i = spool.tile([128, 16], I16, tag="flati")
                nc.vector.tensor_copy(out=flat_i[:], in_=flat_f[:])

                gd = gdpool.tile([128, 256], F32, tag="gd", name="gd")
                nc.gpsimd.ap_gather(
                    out_ap=gd[:].rearrange("p (n d) -> p n d", d=1),
                    in_ap=dect_sb[:].rearrange("p (n d) -> p n d", d=1),
                    idxs_ap=flat_i[:],
                    channels=128, num_elems=NB * NCENT, d=1, num_idxs=256)
                nc.sync.dma_start(out_d[t], gd[:])

            if repeat == 1:
                for t in range(n_tiles):
                    tile_body(t)
            else:
                with tc.For_i(0, repeat, 1):
                    for t in range(n_tiles):
                        tile_body(t)

    nc.compile()
    _NC_CACHE[key] = nc
    return nc


def prep_inputs(k, E_blocks, centroids, decoded_centroids, n_tiles=N_TILES):
    maps = _base_prep(k, E_blocks, centroids, decoded_centroids, n_tiles=n_tiles)
    binc = np.broadcast_to((256.0 * np.arange(16) - C1BIG).astype(np.float32),
                           (128, 16)).copy()
    decf = np.asarray(decoded_centroids, np.float32).reshape(NB * NCENT, BD)
    ch = np.arange(128)
    dect = np.ascontiguousarray(decf[:, ch % 8].T)   # [128, 4096]
    for m in maps:
        del m["iota"]
        m["binc"] = binc
        m["dect"] = dect
        m.pop("dec", None)
    return maps


def unscramble(dev_out, n_tiles=N_TILES):
    """dev_out [n_tiles, 128, 256]: [t, ch=16g+ch', i=16b+p'] ->
    token = t*128 + 16g + p', block = b, d = ch'%8 (take ch'<8)."""
    d = dev_out.reshape(n_tiles, 8, 16, 16, 16)      # [t, g, ch', b, p']
    d = d[:, :, 0:8]                                 # keep one copy of d
    d = d.transpose(0, 1, 4, 3, 2)                   # [t, g, p', b, d]
    return np.ascontiguousarray(d).reshape(n_tiles * 128, 128)


def kernel(k, E_blocks, centroids, decoded_centroids):
    nc = build_nc()
    in_maps = prep_inputs(k, E_blocks, centroids, decoded_centroids)
    res = bass_utils.run_bass_kernel_spmd(nc, in_maps, core_ids=list(range(N_CORES)))
    out = np.concatenate([unscramble(res.results[c]["khat"]) for c in range(N_CORES)],
                         axis=0)
    return out.reshape(B, H, S, DDIM)
